# revision 57
# baseline (speedup 1.0000x reference)
"""Trainium2 Bass kernel for nn_AppearanceTrackletEmb (dense_transformer).

Pure data-parallel: batch B=256 sharded across 8 NeuronCores (32 pairs /
64 independent sequences per core). All weights replicated. Everything
(4 attention layers + pooling + classifier) runs on-chip in one NEFF.

Layout strategy (per core):
  - x is kept channel-major [C, tokens] on chip; the host pre-transposes
    the layer-0 input to [CIN_pad, tokens] (bf16) so no on-chip transposes
    are ever needed.
  - Attention uses the S' = k^T q orientation: S'[m,l]. The softmax
    denominator Z = ones^T @ E is computed with a ones-matmul whose
    stationary operand is [65, 128], which broadcasts Z across all 128
    partitions; normalization is deferred to the out2 epilogue (a DVE
    multiply by 1/Z) so the PE never waits on the softmax chain.
  - V is produced token-major over 128-token tiles (full PE width), then
    re-partitioned into per-sequence [65, C] tiles with SBUF->SBUF DMAs.
  - Biases: layer-0 biases ride in an augmented ones-row of x/W; layer
    1-3 q/k biases fuse into the PSUM->SBUF activation (per-partition);
    layer 1-3 v biases use a rank-1 ones matmul (skipped when zero).
  - The classifier and the last layer's epilogue run in fp32.
"""

import sys

if "/opt/trn_rl_repo" not in sys.path:
    sys.path.insert(0, "/opt/trn_rl_repo")

from contextlib import ExitStack

import ml_dtypes
import numpy as np

import concourse.bass as bass
import concourse.tile as tile
from concourse import bacc, mybir
from concourse.bass_utils import run_bass_kernel_spmd

BF16 = mybir.dt.bfloat16
F32 = mybir.dt.float32
FP8 = mybir.dt.float8e4
DR = mybir.MatmulPerfMode.DoubleRow
AF = mybir.ActivationFunctionType
ALU = mybir.AluOpType
AX = mybir.AxisListType

C = 512
L = 65
CIN = 2053
SCALE = 1.0 / float(np.sqrt(C))
N_CORES = 8


def build_graph(n_seq=64, G=8, KI=17, vbias=True, use_m=True, use_lrelu=False,
                fp8_qk0=True):
    """Build the per-core Bass graph.

    n_seq: sequences per core (left+right), G: sequences per chunk,
    KI: number of 128-row k-tiles for the (padded) input channel dim,
    vbias: emit the rank-1 v-bias matmuls (skip when biases are zero),
    use_m: zero-qk-bias specialization for layers 1..3 -- S = q^T k
      = x^T (Wq^T Wk) x, so a single projection g = (Wk^T Wq) x replaces
      both q and k (k := x itself in the S' matmul),
    use_lrelu: fused LeakyRelu on the scalar engine for the v epilogue
      (costs an ACT table swap against Exp -- keep off),
    fp8_qk0: layer-0 q/k projections in fp8e4 DoubleRow (K=256/matmul).
      Weights are pre-scaled x64 on the host; epilogues scale by 1/64.
      v0 stays bf16 (fp8 Wv0 error does not attenuate through softmax).
    """
    CINP = KI * 128
    T = G * L          # tokens per chunk
    H = T // 2         # free-dim half (matmul N and PSUM bank limit)
    GH = G // 2        # sequences per half
    n_chunks = n_seq // G
    n_pairs = n_seq // 2
    CT = C // 128      # 4 channel tiles
    NTT = (T + 127) // 128          # 128-token tiles per chunk (for v)
    tts = [(i * 128, min(128, T - i * 128)) for i in range(NTT)]
    KI8 = (KI + 1) // 2             # 256-row DoubleRow k-tiles for layer 0
    T8 = (T + 15) // 16 * 16        # fp8 x free-dim stride (16B aligned)
    assert n_seq % G == 0 and G % 2 == 0
    assert H <= 512

    nc = bacc.Bacc(None, num_swdge_queues=4)

    xT_d = nc.dram_tensor("xT", [n_chunks, 128, KI, T], BF16, kind="ExternalInput")
    w0T_d = nc.dram_tensor("w0T", [3, 128, KI, C], BF16, kind="ExternalInput")
    if fp8_qk0:
        xT8_d = nc.dram_tensor("xT8", [n_chunks, 128, 2 * KI8, T8], FP8,
                               kind="ExternalInput")
        w0qk8_d = nc.dram_tensor("w0qk8", [2, 128, 2 * KI8, C], FP8,
                                 kind="ExternalInput")
    if use_m:
        # mT[ly] = Wq[ly]^T @ Wk[ly] pre-transposed for the stationary slot
        mT_d = nc.dram_tensor("mT", [3, 128, CT, C], BF16, kind="ExternalInput")
    else:
        wqT_d = nc.dram_tensor("wqT", [3, 128, CT, C], BF16, kind="ExternalInput")
        wkT_d = nc.dram_tensor("wkT", [3, 128, CT, C], BF16, kind="ExternalInput")
    wvT_d = nc.dram_tensor("wvT", [3, 128, CT, C], BF16, kind="ExternalInput")
    bqk_d = nc.dram_tensor("bqk", [128, CT, 6], F32, kind="ExternalInput")
    bvT_d = nc.dram_tensor("bvT", [3, C], BF16, kind="ExternalInput")
    w1T_d = nc.dram_tensor("w1T", [128, 8, 256], F32, kind="ExternalInput")
    b1_d = nc.dram_tensor("b1", [128, 2], F32, kind="ExternalInput")
    w2T_d = nc.dram_tensor("w2T", [256, 2], F32, kind="ExternalInput")
    b2_d = nc.dram_tensor("b2", [2, 1], F32, kind="ExternalInput")
    out_d = nc.dram_tensor("out", [2, n_pairs], F32, kind="ExternalOutput")

    with tile.TileContext(nc) as tc, ExitStack() as ctx:
        wpool = ctx.enter_context(tc.tile_pool(name="w", bufs=1))
        xpool = ctx.enter_context(tc.tile_pool(name="xt", bufs=2))
        apool = ctx.enter_context(tc.tile_pool(name="act", bufs=2))
        xlpool = ctx.enter_context(tc.tile_pool(name="xl", bufs=2))
        ps_pool = ctx.enter_context(tc.tile_pool(name="ps", bufs=1, space="PSUM"))

        # --- startup-critical DMAs, in need-order, spread over queues ---
        xt0 = xpool.tile([128, KI, T], BF16, tag="xt")
        # v projections run first, so load v weights + chunk-0 x first
        # interleave v-weights and chunk-0 x in k-tile quarters so the
        # first v matmuls can start as soon as their k-tiles land
        _qs = [(i * KI // 4, (i + 1) * KI // 4) for i in range(4)]
        _qs = [(a, b) for a, b in _qs if b > a]
        if fp8_qk0:
            # weights on gpsimd queue, x on sync queue, fp8 pair on vector:
            # three queues stream in parallel so the first v matmul can
            # start after ~one k-tile quarter instead of the full load
            # the first qk pair gates PE start; HBM bandwidth is shared by
            # all queues, so split each input into halves and order them so
            # the first (j=0, ct=0) block's gate is ~2.4MB, not the full 4.8
            w0qk8_sb = wpool.tile([128, 2, 2 * KI8, C], FP8)
            xt8_0 = xpool.tile([128, 2 * KI8, T8], FP8, tag="xt8")
            xt8_1 = None
            hk = (KI8 // 2 + 1) * 2  # row split aligned to DoubleRow pairs
            nc.scalar.dma_start(w0qk8_sb[:, 0], w0qk8_d[0])
            nc.sync.dma_start(xt8_0[:, :hk], xT8_d[0, :, :hk])
            if n_seq // G > 1:  # partner chunk of the first fp8 qk pair
                xt8_1 = xpool.tile([128, 2 * KI8, T8], FP8, tag="xt8",
                                   name="xt8_1")
                nc.gpsimd.dma_start(xt8_1[:, :hk], xT8_d[1, :, :hk])
                nc.gpsimd.dma_start(xt8_1[:, hk:], xT8_d[1, :, hk:])
            nc.sync.dma_start(xt8_0[:, hk:], xT8_d[0, :, hk:])
            nc.scalar.dma_start(w0qk8_sb[:, 1], w0qk8_d[1])
            w0v_sb = wpool.tile([128, KI, C], BF16)
            for a, b in _qs:
                nc.gpsimd.dma_start(w0v_sb[:, a:b], w0T_d[2, :, a:b])
                nc.sync.dma_start(xt0[:, a:b], xT_d[0, :, a:b])
            w0v_ap = lambda ki: w0v_sb[:, ki, :]
        else:
            w0T_sb = wpool.tile([128, 3, KI, C], BF16)
            for a, b in _qs:
                nc.sync.dma_start(w0T_sb[:, 2, a:b], w0T_d[2, :, a:b])
                nc.sync.dma_start(xt0[:, a:b], xT_d[0, :, a:b])
            nc.sync.dma_start(w0T_sb[:, 0], w0T_d[0])
            nc.sync.dma_start(w0T_sb[:, 1], w0T_d[1])
            w0v_ap = lambda ki: w0T_sb[:, 2, ki, :]

        # --- remaining weights on the scalar HWDGE queue (off sync) ---
        if use_m:
            mT_sb = wpool.tile([128, 3, CT, C], BF16)
            nc.scalar.dma_start(mT_sb[:], mT_d.rearrange("n p a c -> p n a c"))
        else:
            wqT_sb = wpool.tile([128, 3, CT, C], BF16)
            nc.scalar.dma_start(wqT_sb[:], wqT_d.rearrange("n p a c -> p n a c"))
            wkT_sb = wpool.tile([128, 3, CT, C], BF16)
            nc.scalar.dma_start(wkT_sb[:], wkT_d.rearrange("n p a c -> p n a c"))
        wvT_sb = wpool.tile([128, 3, CT, C], BF16)
        nc.scalar.dma_start(wvT_sb[:], wvT_d.rearrange("n p a c -> p n a c"))
        bqk_sb = wpool.tile([128, CT, 6], F32)
        nc.scalar.dma_start(bqk_sb[:], bqk_d[:])
        bvT_sb = wpool.tile([1, 3, C], BF16)
        nc.scalar.dma_start(bvT_sb[:], bvT_d[None, :, :])
        w1T_sb = wpool.tile([128, 8, 256], F32)
        nc.scalar.dma_start(w1T_sb[:], w1T_d[:])
        b1_sb = wpool.tile([128, 2], F32)
        nc.scalar.dma_start(b1_sb[:], b1_d[:])
        w2T_sb = wpool.tile([128, 2, 2], F32)
        nc.scalar.dma_start(w2T_sb[:], w2T_d.rearrange("(a p) m -> p a m", p=128))
        b2_sb = wpool.tile([2, 1], F32)
        nc.scalar.dma_start(b2_sb[:], b2_d[:])

        onesZ = wpool.tile([65, 128], BF16)   # Z-broadcast matmul stationary
        nc.vector.memset(onesZ[:], 1.0)
        ones1 = wpool.tile([1, 128], BF16)    # rank-1 v-bias stationary
        nc.vector.memset(ones1[:], 1.0)

        P_sb = wpool.tile([128, CT, n_seq], F32)
        h2_sb = wpool.tile([128, 2, n_pairs], F32)
        y_sb = wpool.tile([2, n_pairs], F32)

        def proj_v_tiles(xt_or_x, nki, wv_ap, ly, name="v"):
            """Token-major v over 128-token tiles + re-partition to [65,G,C].

            xt_or_x: channel-major input [128, nki, T]; wv_ap(ki) -> [128, C].
            Returns (vT, [tile emitters]) so the caller can interleave the
            per-tile matmul emission with other work (software pipelining).
            """
            vstage = apool.tile([128, NTT, C], BF16, tag="vs", bufs=3,
                                name=f"vs_{name}")
            vT = apool.tile([65, G, C], BF16, tag="vt", name=f"vt_{name}")

            def emit(tt):
                off, tw = tts[tt]
                ps = ps_pool.tile([128, C], F32, tag="pjv", bufs=2)
                for ki in range(nki):
                    nc.tensor.matmul(
                        ps[0:tw, :],
                        xt_or_x[:, ki, off:off + tw],
                        wv_ap(ki),
                        start=(ki == 0),
                        stop=(ki == nki - 1 and not (vbias and ly >= 0)),
                    )
                if vbias and ly >= 0:  # rank-1 bias: ones(tw) x bv
                    nc.tensor.matmul(ps[0:tw, :], ones1[0:1, 0:tw],
                                     bvT_sb[:, ly, :], start=False, stop=True)
                if use_lrelu:
                    nc.scalar.activation(vstage[0:tw, tt, :], ps[0:tw, :],
                                         AF.Lrelu, scale=1.0, alpha=0.01)
                else:
                    vr = apool.tile([128, C], F32, tag="vr")
                    nc.scalar.activation(vr[0:tw, :], ps[0:tw, :], AF.Relu,
                                         scale=0.99)
                    nc.vector.scalar_tensor_tensor(
                        vstage[0:tw, tt, :], ps[0:tw, :], 0.01, vr[0:tw, :],
                        op0=ALU.mult, op1=ALU.add)
                # re-partition each sequence as soon as its last source
                # tile is evacuated; alternate queues so the 2D SBUF->SBUF
                # descriptors don't serialize on one DGE queue
                for s in range(G):
                    t0 = s * L
                    a, r = divmod(t0, 128)
                    n1 = min(L, 128 - r)
                    last_tile = a if n1 == L else a + 1
                    if last_tile != tt:
                        continue
                    eng = nc.sync if s % 2 == 0 else nc.gpsimd
                    eng.dma_start(vT[0:n1, s, :], vstage[r:r + n1, a, :])
                    if n1 < L:
                        eng.dma_start(vT[n1:L, s, :],
                                      vstage[0:L - n1, a + 1, :])

            return vT, [(lambda tt=tt: emit(tt)) for tt in range(NTT)]

        def proj_v(xt_or_x, nki, wv_ap, ly, name="v"):
            vT, emitters = proj_v_tiles(xt_or_x, nki, wv_ap, ly, name)
            for fn in emitters:
                fn()
            return vT

        def proj_qk0_pair(x8a, x8b):
            """Layer-0 q/k for TWO chunks in fp8 DoubleRow.

            Four N=260 matmuls stream off each 256-column LDWEIGHTS (which
            cannot overlap matmuls in DoubleRow mode -- it occupies both
            weight buffers), so the LDW cost is amortized 4 ways.
            """
            tiles = {}
            for j in range(2):  # 0=q, 1=k (channel-major [C, T])
                tiles[j, 0] = apool.tile([128, CT, T], BF16, tag=f"qk{j}",
                                         name=f"qk{j}_a")
                tiles[j, 1] = apool.tile([128, CT, T], BF16, tag=f"qk{j}",
                                         name=f"qk{j}_b")

            def emit(j, ct):
                dst_a, dst_b = tiles[j, 0], tiles[j, 1]
                work = [(x8a, dst_a, 0), (x8a, dst_a, 1),
                        (x8b, dst_b, 0), (x8b, dst_b, 1)]
                pss = [ps_pool.tile([128, H], F32, tag="pj", bufs=4,
                                    name=f"psqk{j}_{ct}_{i}")
                       for i in range(4)]
                for kt in range(KI8):
                    for (x8, _, h), ps in zip(work, pss):
                        nc.tensor.matmul(
                            ps[:],
                            w0qk8_sb[:, j, 2 * kt:2 * kt + 2,
                                     ct * 128:(ct + 1) * 128],
                            x8[:, 2 * kt:2 * kt + 2, h * H:(h + 1) * H],
                            start=(kt == 0),
                            stop=(kt == KI8 - 1),
                            perf_mode=DR,
                        )
                for (_, dst, h), ps in zip(work, pss):
                    # both q and k evac on ACT: keeps the DVE FIFO free for
                    # the latency-critical v epilogues / out epilogues
                    nc.scalar.activation(
                        dst[:, ct, h * H:(h + 1) * H], ps[:],
                        AF.Copy, scale=1.0 / 64)

            emitters = [(lambda j=j, ct=ct: emit(j, ct))
                        for j in range(2) for ct in range(CT)]
            return ((tiles[0, 0], tiles[1, 0]), (tiles[0, 1], tiles[1, 1]),
                    emitters)

        def proj_layer0(xt):
            vT = proj_v(xt, KI, w0v_ap, -1)
            qk = []
            for j in range(2):  # 0=q, 1=k (channel-major [C, T])
                dst = apool.tile([128, CT, T], BF16, tag=f"qk{j}")
                for ct in range(CT):
                    for h in range(2):
                        ps = ps_pool.tile([128, H], F32, tag="pj", bufs=4)
                        for ki in range(KI):
                            nc.tensor.matmul(
                                ps[:],
                                w0T_sb[:, j, ki, ct * 128:(ct + 1) * 128],
                                xt[:, ki, h * H:(h + 1) * H],
                                start=(ki == 0),
                                stop=(ki == KI - 1),
                            )
                        if j == 0:
                            nc.scalar.activation(dst[:, ct, h * H:(h + 1) * H],
                                                 ps[:], AF.Copy)
                        else:
                            nc.vector.tensor_copy(dst[:, ct, h * H:(h + 1) * H],
                                                  ps[:])
                qk.append(dst)
            return qk[0], qk[1], vT

        def proj_layer(x, ly):
            vT = proj_v(x, CT, lambda ki: wvT_sb[:, ly, ki, :], ly)
            if use_m:
                # g = (Wk^T Wq) x; S'[m,l] = sum_c x[c,m] g[c,l]
                g = apool.tile([128, CT, T], BF16, tag="g")
                for ct in range(CT):
                    for h in range(2):
                        ps = ps_pool.tile([128, H], F32, tag="pj", bufs=4)
                        for ki in range(CT):
                            nc.tensor.matmul(
                                ps[:],
                                mT_sb[:, ly, ki, ct * 128:(ct + 1) * 128],
                                x[:, ki, h * H:(h + 1) * H],
                                start=(ki == 0),
                                stop=(ki == CT - 1),
                            )
                        nc.scalar.activation(g[:, ct, h * H:(h + 1) * H],
                                             ps[:], AF.Copy)
                return g, x, vT
            qk = []
            for j, w in enumerate((wqT_sb, wkT_sb)):
                dst = apool.tile([128, CT, T], BF16, tag=f"qk{j}")
                for ct in range(CT):
                    for h in range(2):
                        ps = ps_pool.tile([128, H], F32, tag="pj", bufs=4)
                        for ki in range(CT):
                            nc.tensor.matmul(
                                ps[:],
                                w[:, ly, ki, ct * 128:(ct + 1) * 128],
                                x[:, ki, h * H:(h + 1) * H],
                                start=(ki == 0),
                                stop=(ki == CT - 1),
                            )
                        col = j * 3 + ly
                        if j == 0:
                            nc.scalar.activation(
                                dst[:, ct, h * H:(h + 1) * H], ps[:],
                                AF.Identity, bias=bqk_sb[:, ct, col:col + 1],
                                scale=1.0,
                            )
                        else:
                            nc.vector.tensor_scalar_add(
                                dst[:, ct, h * H:(h + 1) * H], ps[:],
                                bqk_sb[:, ct, col:col + 1],
                            )
                qk.append(dst)
            return qk[0], qk[1], vT

        SG = min(4, G)          # sequences per S' psum group
        n_sg = G // SG

        def softmax_parts(q, k):
            E = apool.tile([65, T], BF16, tag="E")
            for g in range(n_sg):
                ps = ps_pool.tile([65, SG * L], F32, tag="sp", bufs=2)
                for s4 in range(SG):
                    s = g * SG + s4
                    for ct in range(CT):
                        nc.tensor.matmul(
                            ps[:, s4 * L:(s4 + 1) * L],
                            k[:, ct, s * L:(s + 1) * L],
                            q[:, ct, s * L:(s + 1) * L],
                            start=(ct == 0),
                            stop=(ct == CT - 1),
                        )
                nc.scalar.activation(E[:65, g * SG * L:(g + 1) * SG * L], ps[:],
                                     AF.Exp, scale=SCALE)
            zr = apool.tile([128, T], F32, tag="zr")
            for h in range(2):
                psz = ps_pool.tile([128, H], F32, tag="pj", bufs=4)
                nc.tensor.matmul(psz[:], onesZ[:], E[:65, h * H:(h + 1) * H],
                                 start=True, stop=True)
                nc.vector.reciprocal_approx_fast(zr[:, h * H:(h + 1) * H], psz[:])
            return E, zr

        def attention(q, k, vT, x_prev, fill=None):
            E, zr = softmax_parts(q, k)
            if fill is not None:
                fill()  # cover the S'->exp->zr chain latency before out
            xn = xlpool.tile([128, CT, T], BF16, tag="x", bufs=2)
            # h outer: the first half's out matmuls only need seqs 0..GH-1,
            # whose vT repartition DMAs land first
            for h in range(2):
                for ct in range(CT):
                    ps = ps_pool.tile([128, H], F32, tag="pj", bufs=4)
                    for s4 in range(GH):
                        s = h * GH + s4
                        nc.tensor.matmul(
                            ps[:, s4 * L:(s4 + 1) * L],
                            vT[:65, s, ct * 128:(ct + 1) * 128],
                            E[:65, s * L:(s + 1) * L],
                            start=True,
                            stop=True,
                        )
                    dst = xn[:, ct, h * H:(h + 1) * H]
                    if x_prev is None:
                        nc.vector.tensor_tensor(dst, ps[:], zr[:, h * H:(h + 1) * H],
                                                op=ALU.mult)
                    else:
                        nc.vector.tensor_tensor(ps[:], ps[:], zr[:, h * H:(h + 1) * H],
                                                op=ALU.mult)
                        nc.vector.tensor_add(dst, ps[:],
                                             x_prev[:, ct, h * H:(h + 1) * H])
            return xn

        def attention_last(q, k, vT, x_prev, c):
            # mean over l commutes with out2+residual: pool the attention
            # weights instead of materializing x3.
            E, zr = softmax_parts(q, k)
            Ew = apool.tile([65, T], F32, tag="Ew")
            nc.vector.tensor_tensor(Ew[:65, :], E[:65, :], zr[0:65, :],
                                    op=ALU.mult)
            wf = apool.tile([65, G], F32, tag="wredf")
            nc.vector.tensor_reduce(
                wf[:65, :], Ew[:65, :].rearrange("p (s l) -> p s l", l=L),
                axis=AX.X, op=ALU.add)
            w = apool.tile([65, G], BF16, tag="wred")
            nc.vector.tensor_copy(w[:65, :], wf[:65, :])
            for ct in range(CT):
                ps = ps_pool.tile([128, G], F32, tag="pj", bufs=4)
                for s in range(G):
                    nc.tensor.matmul(ps[:, s:s + 1],
                                     vT[:65, s, ct * 128:(ct + 1) * 128],
                                     w[:65, s:s + 1], start=True, stop=True)
                red = apool.tile([128, G], F32, tag="red")
                nc.vector.tensor_reduce(
                    red[:], x_prev[:, ct, :].rearrange("p (s l) -> p s l", l=L),
                    axis=AX.X, op=ALU.add)
                tmp = apool.tile([128, G], F32, tag="ptmp")
                nc.vector.tensor_add(tmp[:], ps[:], red[:])
                nc.scalar.activation(P_sb[:, ct, c * G:(c + 1) * G], tmp[:],
                                     AF.Copy, scale=1.0 / L)

        def classifier():
            for mt in range(2):
                ps = ps_pool.tile([128, n_pairs], F32, tag="pj", bufs=4)
                for ki in range(8):
                    rhs = (P_sb[:, ki, 0:n_pairs] if ki < CT
                           else P_sb[:, ki - CT, n_pairs:2 * n_pairs])
                    nc.tensor.matmul(ps[:], w1T_sb[:, ki, mt * 128:(mt + 1) * 128],
                                     rhs, start=(ki == 0), stop=(ki == 7))
                nc.scalar.activation(h2_sb[:, mt, :], ps[:], AF.Relu,
                                     bias=b1_sb[:, mt:mt + 1], scale=1.0)
            ps = ps_pool.tile([2, n_pairs], F32, tag="sp", bufs=2)
            for ki in range(2):
                nc.tensor.matmul(ps[:], w2T_sb[:, ki, :], h2_sb[:, ki, :],
                                 start=(ki == 0), stop=(ki == 1))
            nc.scalar.activation(y_sb[:], ps[:], AF.Identity,
                                 bias=b2_sb[:], scale=1.0)
            nc.sync.dma_start(out_d[:], y_sb[:])

        xts = [None] * n_chunks
        xt8s = [None] * n_chunks
        xts[0] = xt0
        if fp8_qk0:
            xt8s[0] = xt8_0
            xt8s[1] = xt8_1
        qks = {}
        vTs = {}
        fill_q = []  # pending PE filler emitters (next chunk v / pair qk)

        def drain(n):
            for _ in range(min(n, len(fill_q))):
                fill_q.pop(0)()

        for c in range(n_chunks):
            # prefetch next chunk FIRST: the interleaved filler tiles below
            # need xt[c+1]/xt8[c+2] well before this chunk's layer chain
            # finishes, and the scalar queue is otherwise idle mid-loop
            # (sync/gpsimd carry the latency-critical vT repartition DMAs)
            if c + 1 < n_chunks:
                xt = xpool.tile([128, KI, T], BF16, tag="xt",
                                name=f"xt_{c + 1}")
                nc.scalar.dma_start(xt[:], xT_d[c + 1])
                xts[c + 1] = xt
                if fp8_qk0 and c + 2 < n_chunks:
                    xt8 = xpool.tile([128, 2 * KI8, T8], FP8, tag="xt8",
                                     name=f"xt8_{c + 2}")
                    nc.scalar.dma_start(xt8[:], xT8_d[c + 2])
                    xt8s[c + 2] = xt8
            if fp8_qk0:
                if c == 0:
                    # qk pair first: its dense fp8 matmuls cover the v
                    # weight/x DMA latency at startup
                    qks[0], qks[1], qk_em = proj_qk0_pair(xt8s[0], xt8s[1])
                    for fn in qk_em:
                        fn()
                    vTs[0] = proj_v(xts[0], KI, w0v_ap, -1, name="l0c0")
                q, k = qks.pop(c)
                vT = vTs.pop(c)
            else:
                q, k, vT = proj_layer0(xts[c])
            # everything in the NEXT chunk's layer 0 (v tiles; q/k blocks of
            # the next fp8 pair) is independent of this chunk's serial layer
            # chain: queue it and interleave the emission into the layer
            # boundaries to fill PE bubbles.  Queue BEFORE this chunk's l0
            # attention so that at a pair-tail chunk (no dense l0 block) the
            # fill hook can cover the softmax-chain wait.
            if fp8_qk0 and c + 1 < n_chunks:
                vTs[c + 1], vem = proj_v_tiles(xts[c + 1], KI, w0v_ap, -1,
                                               name=f"l0c{c + 1}")
                vem = list(vem)
                if c % 2 == 1 and c + 2 < n_chunks:
                    (qks[c + 1], qks[c + 2],
                     qk_em) = proj_qk0_pair(xt8s[c + 1], xt8s[c + 2])
                    # alternate v tiles and qk blocks in the fill queue
                    mixed = []
                    for i in range(max(len(vem), len(qk_em))):
                        if i < len(vem):
                            mixed.append(vem[i])
                        if i < len(qk_em):
                            mixed.append(qk_em[i])
                    fill_q.extend(mixed)
                else:
                    fill_q.extend(vem)
            # pair-tail chunks start with attention directly (qk was done in
            # the pair block), so the softmax-chain wait is an empty-PE
            # bubble there -- pull one filler into it
            x = attention(q, k, vT, None,
                          fill=(lambda: drain(1)) if c % 2 == 1 else None)
            drain(1)
            for ly in range(2):
                q2, k2, vT2 = proj_layer(x, ly)
                x = attention(q2, k2, vT2, x, fill=lambda: drain(1))
                drain(1)
            q2, k2, vT2 = proj_layer(x, 2)
            attention_last(q2, k2, vT2, x, c)
            drain(len(fill_q))  # rest must land before the next chunk starts
        classifier()

    nc.finalize()
    return nc


def _q8(a):
    return np.clip(np.asarray(a, np.float32), -240.0, 240.0).astype(
        ml_dtypes.float8_e4m3)


def prep_weights(Wq0, bq0, Wk0, bk0, Wv0, bv0, Wq, bq, Wk, bk, Wv, bv,
                 W1, b1, W2, b2, KI=17, use_m=True, fp8_qk0=True):
    """Host-side weight prep shared by all cores."""
    bf = ml_dtypes.bfloat16
    CINP = KI * 128
    w0T = np.zeros((3, CINP, C), np.float32)
    for j, (W_, b_) in enumerate([(Wq0, bq0), (Wk0, bk0), (Wv0, bv0)]):
        w0T[j, :CIN, :] = np.asarray(W_, np.float32).T
        w0T[j, CIN, :] = np.asarray(b_, np.float32)
    w0qk8 = None
    if fp8_qk0:
        # layer-0 q/k weights, x64 and e4m3, padded to 256-row DoubleRow
        # tiles: [2, 128, 2*KI8, C] with (p, ks, c) = 64*W^T[ks*128+p, c]
        KI8 = (KI + 1) // 2
        w8 = np.zeros((2, 2 * KI8 * 128, C), np.float32)
        w8[:, :CINP, :] = w0T[:2] * 64.0
        w0qk8 = _q8(w8.reshape(2, 2 * KI8, 128, C).transpose(0, 2, 1, 3))
    if use_m:
        # lhsT for the fused qk projection: mT[ly] = Wq[ly]^T @ Wk[ly]
        wqT = np.stack([
            (np.asarray(Wq, np.float64)[n].T @ np.asarray(Wk, np.float64)[n])
            .astype(np.float32) for n in range(3)])
    else:
        wqT = np.ascontiguousarray(
            np.transpose(np.asarray(Wq, np.float32), (0, 2, 1)))
    wkT = np.ascontiguousarray(np.transpose(np.asarray(Wk, np.float32), (0, 2, 1)))
    wvT = np.ascontiguousarray(np.transpose(np.asarray(Wv, np.float32), (0, 2, 1)))
    # bqk[p, ct, col]: cols 0..2 = bq layers 1..3, 3..5 = bk layers 1..3
    bqk = np.zeros((128, C // 128, 6), np.float32)
    for ly in range(3):
        bqk[:, :, ly] = np.asarray(bq, np.float32)[ly].reshape(C // 128, 128).T
        bqk[:, :, 3 + ly] = np.asarray(bk, np.float32)[ly].reshape(C // 128, 128).T
    w1T = np.ascontiguousarray(np.asarray(W1, np.float32).T)
    b1h = np.ascontiguousarray(np.asarray(b1, np.float32).reshape(2, 128).T)
    w2T = np.ascontiguousarray(np.asarray(W2, np.float32).T)
    b2h = np.asarray(b2, np.float32).reshape(2, 1)
    KIv = CINP // 128
    CTv = C // 128
    w0T = np.ascontiguousarray(
        w0T.reshape(3, KIv, 128, C).transpose(0, 2, 1, 3))
    wqT = np.ascontiguousarray(
        wqT.reshape(3, CTv, 128, C).transpose(0, 2, 1, 3))
    wkT = np.ascontiguousarray(
        wkT.reshape(3, CTv, 128, C).transpose(0, 2, 1, 3))
    wvT = np.ascontiguousarray(
        wvT.reshape(3, CTv, 128, C).transpose(0, 2, 1, 3))
    w1T = np.ascontiguousarray(w1T.reshape(8, 128, 256).transpose(1, 0, 2))
    out = {
        "w0T": w0T.astype(bf),
        "wvT": wvT.astype(bf),
        "bqk": bqk,
        "bvT": np.asarray(bv, np.float32).astype(bf),
        "w1T": w1T,
        "b1": b1h,
        "w2T": w2T,
        "b2": b2h,
    }
    if use_m:
        out["mT"] = wqT.astype(bf)
    else:
        out["wqT"] = wqT.astype(bf)
        out["wkT"] = wkT.astype(bf)
    if fp8_qk0:
        out["w0qk8"] = w0qk8
    return out


def prep_xT(dl, dr, KI=17, G=8):
    """[n_pairs, L, CIN] left+right -> [n_chunks, 128, KI, G*L] bf16 chunks.

    Sequence order: all left sequences then all right sequences.
    Row CIN is the ones-row that carries layer-0 biases.
    """
    bf = ml_dtypes.bfloat16
    CINP = KI * 128
    n_pairs = dl.shape[0]
    ntok = n_pairs * L
    xT = np.zeros((CINP, 2 * ntok), np.float32)
    xT[:CIN, :ntok] = np.asarray(dl, np.float32).reshape(ntok, CIN).T
    xT[:CIN, ntok:] = np.asarray(dr, np.float32).reshape(ntok, CIN).T
    xT[CIN, :] = 1.0
    T = G * L
    n_chunks = (2 * ntok) // T
    # [CINP, tok] -> [n_chunks, 128, KI, T]
    out = xT.reshape(KI, 128, n_chunks, T).transpose(2, 1, 0, 3)
    return np.ascontiguousarray(out).astype(bf), xT


def prep_xT8(xT, KI=17, G=8):
    """fp8 copy of the token-major input for the DoubleRow q/k path:
    [CINP, tok] -> [n_chunks, 128, 2*KI8, T8] e4m3."""
    KI8 = (KI + 1) // 2
    T = G * L
    T8 = (T + 15) // 16 * 16
    n_chunks = xT.shape[1] // T
    out = np.zeros((2 * KI8 * 128, n_chunks, T8), np.float32)
    out[:xT.shape[0]].reshape(-1, n_chunks, T8)[:, :, :T] = (
        xT.reshape(-1, n_chunks, T))
    out = out.reshape(2 * KI8, 128, n_chunks, T8).transpose(2, 1, 0, 3)
    return _q8(np.ascontiguousarray(out))


def _ensure_ntff_hook():
    """Provide antenv.axon_hooks with a ctypes NTFF profile hook if the
    image's antenv lacks it (bass_utils imports it unguarded when
    trace=True under axon)."""
    try:
        from antenv.axon_hooks import get_axon_ntff_profile_hook  # noqa: F401
        return
    except ImportError:
        pass
    import contextlib
    import ctypes
    import types

    import antenv

    mod = types.ModuleType("antenv.axon_hooks")
    holder = {"hook": None}
    mod.set_axon_ntff_profile_hook = lambda h: holder.update(hook=h)
    mod.get_axon_ntff_profile_hook = lambda: holder["hook"]
    sys.modules["antenv.axon_hooks"] = mod
    antenv.axon_hooks = mod

    so_path = "/opt/axon/libaxon_pjrt.so"
    try:
        lib = ctypes.CDLL(so_path)
    except OSError:
        return
    if not hasattr(lib, "axon_start_nrt_profile"):
        return
    lib.axon_start_nrt_profile.argtypes = [ctypes.POINTER(ctypes.c_int64),
                                           ctypes.c_size_t]
    lib.axon_start_nrt_profile.restype = ctypes.c_int64
    lib.axon_stop_nrt_profile.argtypes = [ctypes.c_char_p]
    lib.axon_stop_nrt_profile.restype = ctypes.c_int64

    @contextlib.contextmanager
    def _hook(output_dir, device_ids):
        import jax

        jax.devices()
        if device_ids:
            ids = (ctypes.c_int64 * len(device_ids))(*device_ids)
            rc = lib.axon_start_nrt_profile(ids, len(device_ids))
        else:
            rc = lib.axon_start_nrt_profile(None, 0)
        if rc != 0:
            raise RuntimeError(f"axon_start_nrt_profile rc={rc}")
        try:
            yield
        finally:
            n = lib.axon_stop_nrt_profile(str(output_dir).encode())
            print(f"ntff profile: {n} file(s) written to {output_dir}",
                  file=sys.stderr)

    holder["hook"] = _hook


_GRAPH_CACHE = {}


def _get_graph(n_seq, G, KI, vbias, use_m, fp8_qk0):
    key = (n_seq, G, KI, vbias, use_m, fp8_qk0)
    if key not in _GRAPH_CACHE:
        _GRAPH_CACHE[key] = build_graph(n_seq=n_seq, G=G, KI=KI, vbias=vbias,
                                        use_m=use_m, fp8_qk0=fp8_qk0)
    return _GRAPH_CACHE[key]


FP8_QK0 = True


def kernel(dataleft, dataright, Wq0, bq0, Wk0, bk0, Wv0, bv0,
           Wq, bq, Wk, bk, Wv, bv, W1, b1, W2, b2):
    import os

    B = dataleft.shape[0]
    per = B // N_CORES
    vbias = bool(np.any(np.asarray(bv)))
    use_m = not (np.any(np.asarray(bq)) or np.any(np.asarray(bk)))
    fp8_qk0 = FP8_QK0
    nc = _get_graph(n_seq=2 * per, G=8, KI=17, vbias=vbias, use_m=use_m,
                    fp8_qk0=fp8_qk0)
    wmap = prep_weights(Wq0, bq0, Wk0, bk0, Wv0, bv0, Wq, bq, Wk, bk, Wv, bv,
                        W1, b1, W2, b2, use_m=use_m, fp8_qk0=fp8_qk0)
    in_maps = []
    for i in range(N_CORES):
        m = dict(wmap)
        m["xT"], xT_full = prep_xT(dataleft[i * per:(i + 1) * per],
                                   dataright[i * per:(i + 1) * per])
        if fp8_qk0:
            m["xT8"] = prep_xT8(xT_full)
        in_maps.append(m)
    trace = bool(int(os.environ.get("KERNEL_TRACE", "0")))
    if trace:
        _ensure_ntff_hook()
    res = run_bass_kernel_spmd(nc, in_maps, core_ids=list(range(N_CORES)),
                               trace=trace)
    if trace and res.exec_time_ns is not None:
        print(f"HW exec time: {res.exec_time_ns} ns")
        kernel.last_exec_time_ns = res.exec_time_ns
        kernel.last_profile = res
    out = np.concatenate([np.ascontiguousarray(r["out"].T) for r in res.results], 0)
    return out.astype(np.float32)



# revision 60
# speedup vs baseline: 1.0231x; 1.0231x over previous
"""Trainium2 Bass kernel for nn_AppearanceTrackletEmb (dense_transformer).

Pure data-parallel: batch B=256 sharded across 8 NeuronCores (32 pairs /
64 independent sequences per core). All weights replicated. Everything
(4 attention layers + pooling + classifier) runs on-chip in one NEFF.

Layout strategy (per core):
  - x is kept channel-major [C, tokens] on chip; the host pre-transposes
    the layer-0 input to [CIN_pad, tokens] (bf16) so no on-chip transposes
    are ever needed.
  - Attention uses the S' = k^T q orientation: S'[m,l]. The softmax
    denominator Z = ones^T @ E is computed with a ones-matmul whose
    stationary operand is [65, 128], which broadcasts Z across all 128
    partitions; normalization is deferred to the out2 epilogue (a DVE
    multiply by 1/Z) so the PE never waits on the softmax chain.
  - V is produced token-major over 128-token tiles (full PE width), then
    re-partitioned into per-sequence [65, C] tiles with SBUF->SBUF DMAs.
  - Biases: layer-0 biases ride in an augmented ones-row of x/W; layer
    1-3 q/k biases fuse into the PSUM->SBUF activation (per-partition);
    layer 1-3 v biases use a rank-1 ones matmul (skipped when zero).
  - The classifier and the last layer's epilogue run in fp32.
"""

import sys

if "/opt/trn_rl_repo" not in sys.path:
    sys.path.insert(0, "/opt/trn_rl_repo")

from contextlib import ExitStack

import ml_dtypes
import numpy as np

import concourse.bass as bass
import concourse.tile as tile
from concourse import bacc, mybir
from concourse.bass_utils import run_bass_kernel_spmd

BF16 = mybir.dt.bfloat16
F32 = mybir.dt.float32
FP8 = mybir.dt.float8e4
DR = mybir.MatmulPerfMode.DoubleRow
AF = mybir.ActivationFunctionType
ALU = mybir.AluOpType
AX = mybir.AxisListType

C = 512
L = 65
CIN = 2053
SCALE = 1.0 / float(np.sqrt(C))
N_CORES = 8


def build_graph(n_seq=64, G=8, KI=17, vbias=True, use_m=True, use_lrelu=False,
                fp8_qk0=True):
    """Build the per-core Bass graph.

    n_seq: sequences per core (left+right), G: sequences per chunk,
    KI: number of 128-row k-tiles for the (padded) input channel dim,
    vbias: emit the rank-1 v-bias matmuls (skip when biases are zero),
    use_m: zero-qk-bias specialization for layers 1..3 -- S = q^T k
      = x^T (Wq^T Wk) x, so a single projection g = (Wk^T Wq) x replaces
      both q and k (k := x itself in the S' matmul),
    use_lrelu: fused LeakyRelu on the scalar engine for the v epilogue
      (costs an ACT table swap against Exp -- keep off),
    fp8_qk0: layer-0 q/k projections in fp8e4 DoubleRow (K=256/matmul).
      Weights are pre-scaled x64 on the host; epilogues scale by 1/64.
      v0 stays bf16 (fp8 Wv0 error does not attenuate through softmax).
    """
    CINP = KI * 128
    T = G * L          # tokens per chunk
    H = T // 2         # free-dim half (matmul N and PSUM bank limit)
    GH = G // 2        # sequences per half
    n_chunks = n_seq // G
    n_pairs = n_seq // 2
    CT = C // 128      # 4 channel tiles
    NTT = (T + 127) // 128          # 128-token tiles per chunk (for v)
    tts = [(i * 128, min(128, T - i * 128)) for i in range(NTT)]
    KI8 = (KI + 1) // 2             # 256-row DoubleRow k-tiles for layer 0
    T8 = (T + 15) // 16 * 16        # fp8 x free-dim stride (16B aligned)
    assert n_seq % G == 0 and G % 2 == 0
    assert H <= 512

    nc = bacc.Bacc(None, num_swdge_queues=4)

    xT_d = nc.dram_tensor("xT", [n_chunks, 128, KI, T], BF16, kind="ExternalInput")
    w0T_d = nc.dram_tensor("w0T", [3, 128, KI, C], BF16, kind="ExternalInput")
    if fp8_qk0:
        xT8_d = nc.dram_tensor("xT8", [n_chunks, 128, 2 * KI8, T8], FP8,
                               kind="ExternalInput")
        w0qk8_d = nc.dram_tensor("w0qk8", [2, 128, 2 * KI8, C], FP8,
                                 kind="ExternalInput")
    if use_m:
        # mT[ly] = Wq[ly]^T @ Wk[ly] pre-transposed for the stationary slot
        mT_d = nc.dram_tensor("mT", [3, 128, CT, C], BF16, kind="ExternalInput")
    else:
        wqT_d = nc.dram_tensor("wqT", [3, 128, CT, C], BF16, kind="ExternalInput")
        wkT_d = nc.dram_tensor("wkT", [3, 128, CT, C], BF16, kind="ExternalInput")
    wvT_d = nc.dram_tensor("wvT", [3, 128, CT, C], BF16, kind="ExternalInput")
    bqk_d = nc.dram_tensor("bqk", [128, CT, 6], F32, kind="ExternalInput")
    bvT_d = nc.dram_tensor("bvT", [3, C], BF16, kind="ExternalInput")
    w1T_d = nc.dram_tensor("w1T", [128, 8, 256], F32, kind="ExternalInput")
    b1_d = nc.dram_tensor("b1", [128, 2], F32, kind="ExternalInput")
    w2T_d = nc.dram_tensor("w2T", [256, 2], F32, kind="ExternalInput")
    b2_d = nc.dram_tensor("b2", [2, 1], F32, kind="ExternalInput")
    out_d = nc.dram_tensor("out", [2, n_pairs], F32, kind="ExternalOutput")

    with tile.TileContext(nc) as tc, ExitStack() as ctx:
        wpool = ctx.enter_context(tc.tile_pool(name="w", bufs=1))
        xpool = ctx.enter_context(tc.tile_pool(name="xt", bufs=2))
        apool = ctx.enter_context(tc.tile_pool(name="act", bufs=2))
        xlpool = ctx.enter_context(tc.tile_pool(name="xl", bufs=2))
        ps_pool = ctx.enter_context(tc.tile_pool(name="ps", bufs=1, space="PSUM"))

        # --- startup-critical DMAs, in need-order, spread over queues ---
        xt0 = xpool.tile([128, KI, T], BF16, tag="xt")
        # v projections run first, so load v weights + chunk-0 x first
        # interleave v-weights and chunk-0 x in k-tile quarters so the
        # first v matmuls can start as soon as their k-tiles land
        _qs = [(i * KI // 4, (i + 1) * KI // 4) for i in range(4)]
        _qs = [(a, b) for a, b in _qs if b > a]
        if fp8_qk0:
            # weights on gpsimd queue, x on sync queue, fp8 pair on vector:
            # three queues stream in parallel so the first v matmul can
            # start after ~one k-tile quarter instead of the full load
            # the first qk pair gates PE start; HBM bandwidth is shared by
            # all queues, so split each input into halves and order them so
            # the first (j=0, ct=0) block's gate is ~2.4MB, not the full 4.8
            w0qk8_sb = wpool.tile([128, 2, 2 * KI8, C], FP8)
            xt8_0 = xpool.tile([128, 2 * KI8, T8], FP8, tag="xt8")
            xt8_1 = None
            hk = (KI8 // 2 + 1) * 2  # row split aligned to DoubleRow pairs
            nc.scalar.dma_start(w0qk8_sb[:, 0], w0qk8_d[0])
            nc.sync.dma_start(xt8_0[:, :hk], xT8_d[0, :, :hk])
            if n_seq // G > 1:  # partner chunk of the first fp8 qk pair
                xt8_1 = xpool.tile([128, 2 * KI8, T8], FP8, tag="xt8",
                                   name="xt8_1")
                nc.gpsimd.dma_start(xt8_1[:, :hk], xT8_d[1, :, :hk])
                nc.gpsimd.dma_start(xt8_1[:, hk:], xT8_d[1, :, hk:])
            nc.sync.dma_start(xt8_0[:, hk:], xT8_d[0, :, hk:])
            nc.scalar.dma_start(w0qk8_sb[:, 1], w0qk8_d[1])
            w0v_sb = wpool.tile([128, KI, C], BF16)
            for a, b in _qs:
                nc.gpsimd.dma_start(w0v_sb[:, a:b], w0T_d[2, :, a:b])
                nc.sync.dma_start(xt0[:, a:b], xT_d[0, :, a:b])
            w0v_ap = lambda ki: w0v_sb[:, ki, :]
        else:
            w0T_sb = wpool.tile([128, 3, KI, C], BF16)
            for a, b in _qs:
                nc.sync.dma_start(w0T_sb[:, 2, a:b], w0T_d[2, :, a:b])
                nc.sync.dma_start(xt0[:, a:b], xT_d[0, :, a:b])
            nc.sync.dma_start(w0T_sb[:, 0], w0T_d[0])
            nc.sync.dma_start(w0T_sb[:, 1], w0T_d[1])
            w0v_ap = lambda ki: w0T_sb[:, 2, ki, :]

        # --- remaining weights on the scalar HWDGE queue (off sync) ---
        if use_m:
            mT_sb = wpool.tile([128, 3, CT, C], BF16)
            nc.scalar.dma_start(mT_sb[:], mT_d.rearrange("n p a c -> p n a c"))
        else:
            wqT_sb = wpool.tile([128, 3, CT, C], BF16)
            nc.scalar.dma_start(wqT_sb[:], wqT_d.rearrange("n p a c -> p n a c"))
            wkT_sb = wpool.tile([128, 3, CT, C], BF16)
            nc.scalar.dma_start(wkT_sb[:], wkT_d.rearrange("n p a c -> p n a c"))
        wvT_sb = wpool.tile([128, 3, CT, C], BF16)
        nc.scalar.dma_start(wvT_sb[:], wvT_d.rearrange("n p a c -> p n a c"))
        bqk_sb = wpool.tile([128, CT, 6], F32)
        nc.scalar.dma_start(bqk_sb[:], bqk_d[:])
        bvT_sb = wpool.tile([1, 3, C], BF16)
        nc.scalar.dma_start(bvT_sb[:], bvT_d[None, :, :])
        w1T_sb = wpool.tile([128, 8, 256], F32)
        nc.scalar.dma_start(w1T_sb[:], w1T_d[:])
        b1_sb = wpool.tile([128, 2], F32)
        nc.scalar.dma_start(b1_sb[:], b1_d[:])
        w2T_sb = wpool.tile([128, 2, 2], F32)
        nc.scalar.dma_start(w2T_sb[:], w2T_d.rearrange("(a p) m -> p a m", p=128))
        b2_sb = wpool.tile([2, 1], F32)
        nc.scalar.dma_start(b2_sb[:], b2_d[:])

        onesZ = wpool.tile([65, 128], BF16)   # Z-broadcast matmul stationary
        nc.vector.memset(onesZ[:], 1.0)
        ones1 = wpool.tile([1, 128], BF16)    # rank-1 v-bias stationary
        nc.vector.memset(ones1[:], 1.0)

        P_sb = wpool.tile([128, CT, n_seq], F32)
        h2_sb = wpool.tile([128, 2, n_pairs], F32)
        y_sb = wpool.tile([2, n_pairs], F32)

        def proj_v_tiles(xt_or_x, nki, wv_ap, ly, name="v"):
            """Token-major v over 128-token tiles + re-partition to [65,G,C].

            xt_or_x: channel-major input [128, nki, T]; wv_ap(ki) -> [128, C].
            Returns (vT, [tile emitters]) so the caller can interleave the
            per-tile matmul emission with other work (software pipelining).
            """
            vstage = apool.tile([128, NTT, C], BF16, tag="vs", bufs=3,
                                name=f"vs_{name}")
            vT = apool.tile([65, G, C], BF16, tag="vt", name=f"vt_{name}")

            def emit(tt):
                off, tw = tts[tt]
                ps = ps_pool.tile([128, C], F32, tag="pjv", bufs=2)
                for ki in range(nki):
                    nc.tensor.matmul(
                        ps[0:tw, :],
                        xt_or_x[:, ki, off:off + tw],
                        wv_ap(ki),
                        start=(ki == 0),
                        stop=(ki == nki - 1 and not (vbias and ly >= 0)),
                    )
                if vbias and ly >= 0:  # rank-1 bias: ones(tw) x bv
                    nc.tensor.matmul(ps[0:tw, :], ones1[0:1, 0:tw],
                                     bvT_sb[:, ly, :], start=False, stop=True)
                if use_lrelu:
                    nc.scalar.activation(vstage[0:tw, tt, :], ps[0:tw, :],
                                         AF.Lrelu, scale=1.0, alpha=0.01)
                else:
                    vr = apool.tile([128, C], F32, tag="vr")
                    nc.scalar.activation(vr[0:tw, :], ps[0:tw, :], AF.Relu,
                                         scale=0.99)
                    nc.vector.scalar_tensor_tensor(
                        vstage[0:tw, tt, :], ps[0:tw, :], 0.01, vr[0:tw, :],
                        op0=ALU.mult, op1=ALU.add)
                # re-partition each sequence as soon as its last source
                # tile is evacuated; alternate queues so the 2D SBUF->SBUF
                # descriptors don't serialize on one DGE queue
                for s in range(G):
                    t0 = s * L
                    a, r = divmod(t0, 128)
                    n1 = min(L, 128 - r)
                    last_tile = a if n1 == L else a + 1
                    if last_tile != tt:
                        continue
                    eng = nc.sync if s % 2 == 0 else nc.gpsimd
                    eng.dma_start(vT[0:n1, s, :], vstage[r:r + n1, a, :])
                    if n1 < L:
                        eng.dma_start(vT[n1:L, s, :],
                                      vstage[0:L - n1, a + 1, :])

            return vT, [(lambda tt=tt: emit(tt)) for tt in range(NTT)]

        def proj_v(xt_or_x, nki, wv_ap, ly, name="v"):
            vT, emitters = proj_v_tiles(xt_or_x, nki, wv_ap, ly, name)
            for fn in emitters:
                fn()
            return vT

        def proj_qk0_pair(x8a, x8b):
            """Layer-0 q/k for TWO chunks in fp8 DoubleRow.

            Four N=260 matmuls stream off each 256-column LDWEIGHTS (which
            cannot overlap matmuls in DoubleRow mode -- it occupies both
            weight buffers), so the LDW cost is amortized 4 ways.
            """
            tiles = {}
            for j in range(2):  # 0=q, 1=k (channel-major [C, T])
                tiles[j, 0] = apool.tile([128, CT, T], BF16, tag=f"qk{j}",
                                         name=f"qk{j}_a")
                tiles[j, 1] = apool.tile([128, CT, T], BF16, tag=f"qk{j}",
                                         name=f"qk{j}_b")

            def emit(j, ct):
                dst_a, dst_b = tiles[j, 0], tiles[j, 1]
                work = [(x8a, dst_a, 0), (x8a, dst_a, 1),
                        (x8b, dst_b, 0), (x8b, dst_b, 1)]
                pss = [ps_pool.tile([128, H], F32, tag="pj", bufs=4,
                                    name=f"psqk{j}_{ct}_{i}")
                       for i in range(4)]
                for kt in range(KI8):
                    for (x8, _, h), ps in zip(work, pss):
                        nc.tensor.matmul(
                            ps[:],
                            w0qk8_sb[:, j, 2 * kt:2 * kt + 2,
                                     ct * 128:(ct + 1) * 128],
                            x8[:, 2 * kt:2 * kt + 2, h * H:(h + 1) * H],
                            start=(kt == 0),
                            stop=(kt == KI8 - 1),
                            perf_mode=DR,
                        )
                for (_, dst, h), ps in zip(work, pss):
                    # both q and k evac on ACT: keeps the DVE FIFO free for
                    # the latency-critical v epilogues / out epilogues
                    nc.scalar.activation(
                        dst[:, ct, h * H:(h + 1) * H], ps[:],
                        AF.Copy, scale=1.0 / 64)

            emitters = [(lambda j=j, ct=ct: emit(j, ct))
                        for j in range(2) for ct in range(CT)]
            return ((tiles[0, 0], tiles[1, 0]), (tiles[0, 1], tiles[1, 1]),
                    emitters)

        def proj_layer0(xt):
            vT = proj_v(xt, KI, w0v_ap, -1)
            qk = []
            for j in range(2):  # 0=q, 1=k (channel-major [C, T])
                dst = apool.tile([128, CT, T], BF16, tag=f"qk{j}")
                for ct in range(CT):
                    for h in range(2):
                        ps = ps_pool.tile([128, H], F32, tag="pj", bufs=4)
                        for ki in range(KI):
                            nc.tensor.matmul(
                                ps[:],
                                w0T_sb[:, j, ki, ct * 128:(ct + 1) * 128],
                                xt[:, ki, h * H:(h + 1) * H],
                                start=(ki == 0),
                                stop=(ki == KI - 1),
                            )
                        if j == 0:
                            nc.scalar.activation(dst[:, ct, h * H:(h + 1) * H],
                                                 ps[:], AF.Copy)
                        else:
                            nc.vector.tensor_copy(dst[:, ct, h * H:(h + 1) * H],
                                                  ps[:])
                qk.append(dst)
            return qk[0], qk[1], vT

        def proj_layer(x, ly):
            vT = proj_v(x, CT, lambda ki: wvT_sb[:, ly, ki, :], ly)
            if use_m:
                # g = (Wk^T Wq) x; S'[m,l] = sum_c x[c,m] g[c,l]
                g = apool.tile([128, CT, T], BF16, tag="g")
                for ct in range(CT):
                    for h in range(2):
                        ps = ps_pool.tile([128, H], F32, tag="pj", bufs=4)
                        for ki in range(CT):
                            nc.tensor.matmul(
                                ps[:],
                                mT_sb[:, ly, ki, ct * 128:(ct + 1) * 128],
                                x[:, ki, h * H:(h + 1) * H],
                                start=(ki == 0),
                                stop=(ki == CT - 1),
                            )
                        nc.scalar.activation(g[:, ct, h * H:(h + 1) * H],
                                             ps[:], AF.Copy)
                return g, x, vT
            qk = []
            for j, w in enumerate((wqT_sb, wkT_sb)):
                dst = apool.tile([128, CT, T], BF16, tag=f"qk{j}")
                for ct in range(CT):
                    for h in range(2):
                        ps = ps_pool.tile([128, H], F32, tag="pj", bufs=4)
                        for ki in range(CT):
                            nc.tensor.matmul(
                                ps[:],
                                w[:, ly, ki, ct * 128:(ct + 1) * 128],
                                x[:, ki, h * H:(h + 1) * H],
                                start=(ki == 0),
                                stop=(ki == CT - 1),
                            )
                        col = j * 3 + ly
                        if j == 0:
                            nc.scalar.activation(
                                dst[:, ct, h * H:(h + 1) * H], ps[:],
                                AF.Identity, bias=bqk_sb[:, ct, col:col + 1],
                                scale=1.0,
                            )
                        else:
                            nc.vector.tensor_scalar_add(
                                dst[:, ct, h * H:(h + 1) * H], ps[:],
                                bqk_sb[:, ct, col:col + 1],
                            )
                qk.append(dst)
            return qk[0], qk[1], vT

        SG = min(4, G)          # sequences per S' psum group
        n_sg = G // SG

        def softmax_parts(q, k):
            E = apool.tile([65, T], BF16, tag="E")
            for g in range(n_sg):
                ps = ps_pool.tile([65, SG * L], F32, tag="sp", bufs=2)
                for s4 in range(SG):
                    s = g * SG + s4
                    for ct in range(CT):
                        nc.tensor.matmul(
                            ps[:, s4 * L:(s4 + 1) * L],
                            k[:, ct, s * L:(s + 1) * L],
                            q[:, ct, s * L:(s + 1) * L],
                            start=(ct == 0),
                            stop=(ct == CT - 1),
                        )
                nc.scalar.activation(E[:65, g * SG * L:(g + 1) * SG * L], ps[:],
                                     AF.Exp, scale=SCALE)
            zr = apool.tile([128, T], F32, tag="zr")
            for h in range(2):
                psz = ps_pool.tile([128, H], F32, tag="pj", bufs=4)
                nc.tensor.matmul(psz[:], onesZ[:], E[:65, h * H:(h + 1) * H],
                                 start=True, stop=True)
                nc.vector.reciprocal_approx_fast(zr[:, h * H:(h + 1) * H], psz[:])
            return E, zr

        def attention(q, k, vT, x_prev, fill=None):
            E, zr = softmax_parts(q, k)
            if fill is not None:
                fill()  # cover the S'->exp->zr chain latency before out
            xn = xlpool.tile([128, CT, T], BF16, tag="x", bufs=2)
            # h outer: the first half's out matmuls only need seqs 0..GH-1,
            # whose vT repartition DMAs land first
            for h in range(2):
                for ct in range(CT):
                    ps = ps_pool.tile([128, H], F32, tag="pj", bufs=4)
                    for s4 in range(GH):
                        s = h * GH + s4
                        nc.tensor.matmul(
                            ps[:, s4 * L:(s4 + 1) * L],
                            vT[:65, s, ct * 128:(ct + 1) * 128],
                            E[:65, s * L:(s + 1) * L],
                            start=True,
                            stop=True,
                        )
                    dst = xn[:, ct, h * H:(h + 1) * H]
                    if x_prev is None:
                        nc.vector.tensor_tensor(dst, ps[:], zr[:, h * H:(h + 1) * H],
                                                op=ALU.mult)
                    else:
                        nc.vector.tensor_tensor(ps[:], ps[:], zr[:, h * H:(h + 1) * H],
                                                op=ALU.mult)
                        nc.vector.tensor_add(dst, ps[:],
                                             x_prev[:, ct, h * H:(h + 1) * H])
            return xn

        def attention_last(q, k, vT, x_prev, c, fill=None):
            # mean over l commutes with out2+residual: pool the attention
            # weights instead of materializing x3.
            E, zr = softmax_parts(q, k)
            if fill is not None:
                fill()  # the Ew->wf->w DVE chain leaves the PE empty here
            Ew = apool.tile([65, T], F32, tag="Ew")
            nc.vector.tensor_tensor(Ew[:65, :], E[:65, :], zr[0:65, :],
                                    op=ALU.mult)
            wf = apool.tile([65, G], F32, tag="wredf")
            nc.vector.tensor_reduce(
                wf[:65, :], Ew[:65, :].rearrange("p (s l) -> p s l", l=L),
                axis=AX.X, op=ALU.add)
            w = apool.tile([65, G], BF16, tag="wred")
            nc.vector.tensor_copy(w[:65, :], wf[:65, :])
            for ct in range(CT):
                ps = ps_pool.tile([128, G], F32, tag="pj", bufs=4)
                for s in range(G):
                    nc.tensor.matmul(ps[:, s:s + 1],
                                     vT[:65, s, ct * 128:(ct + 1) * 128],
                                     w[:65, s:s + 1], start=True, stop=True)
                red = apool.tile([128, G], F32, tag="red")
                nc.vector.tensor_reduce(
                    red[:], x_prev[:, ct, :].rearrange("p (s l) -> p s l", l=L),
                    axis=AX.X, op=ALU.add)
                tmp = apool.tile([128, G], F32, tag="ptmp")
                nc.vector.tensor_add(tmp[:], ps[:], red[:])
                nc.scalar.activation(P_sb[:, ct, c * G:(c + 1) * G], tmp[:],
                                     AF.Copy, scale=1.0 / L)

        def classifier():
            for mt in range(2):
                ps = ps_pool.tile([128, n_pairs], F32, tag="pj", bufs=4)
                for ki in range(8):
                    rhs = (P_sb[:, ki, 0:n_pairs] if ki < CT
                           else P_sb[:, ki - CT, n_pairs:2 * n_pairs])
                    nc.tensor.matmul(ps[:], w1T_sb[:, ki, mt * 128:(mt + 1) * 128],
                                     rhs, start=(ki == 0), stop=(ki == 7))
                nc.scalar.activation(h2_sb[:, mt, :], ps[:], AF.Relu,
                                     bias=b1_sb[:, mt:mt + 1], scale=1.0)
            ps = ps_pool.tile([2, n_pairs], F32, tag="sp", bufs=2)
            for ki in range(2):
                nc.tensor.matmul(ps[:], w2T_sb[:, ki, :], h2_sb[:, ki, :],
                                 start=(ki == 0), stop=(ki == 1))
            nc.scalar.activation(y_sb[:], ps[:], AF.Identity,
                                 bias=b2_sb[:], scale=1.0)
            nc.sync.dma_start(out_d[:], y_sb[:])

        xts = [None] * n_chunks
        xt8s = [None] * n_chunks
        xts[0] = xt0
        if fp8_qk0:
            xt8s[0] = xt8_0
            xt8s[1] = xt8_1
        qks = {}
        vTs = {}
        fill_q = []  # pending PE filler emitters (next chunk v / pair qk)

        def drain(n):
            for _ in range(min(n, len(fill_q))):
                fill_q.pop(0)()

        for c in range(n_chunks):
            # prefetch next chunk FIRST: the interleaved filler tiles below
            # need xt[c+1]/xt8[c+2] well before this chunk's layer chain
            # finishes, and the scalar queue is otherwise idle mid-loop
            # (sync/gpsimd carry the latency-critical vT repartition DMAs)
            if c + 1 < n_chunks:
                xt = xpool.tile([128, KI, T], BF16, tag="xt",
                                name=f"xt_{c + 1}")
                nc.scalar.dma_start(xt[:], xT_d[c + 1])
                xts[c + 1] = xt
                if fp8_qk0 and c + 2 < n_chunks:
                    xt8 = xpool.tile([128, 2 * KI8, T8], FP8, tag="xt8",
                                     name=f"xt8_{c + 2}")
                    nc.scalar.dma_start(xt8[:], xT8_d[c + 2])
                    xt8s[c + 2] = xt8
            if fp8_qk0:
                if c == 0:
                    # qk pair first: its dense fp8 matmuls cover the v
                    # weight/x DMA latency at startup
                    qks[0], qks[1], qk_em = proj_qk0_pair(xt8s[0], xt8s[1])
                    for fn in qk_em:
                        fn()
                    vTs[0] = proj_v(xts[0], KI, w0v_ap, -1, name="l0c0")
                q, k = qks.pop(c)
                vT = vTs.pop(c)
            else:
                q, k, vT = proj_layer0(xts[c])
            x = attention(q, k, vT, None)
            # everything in the NEXT chunk's layer 0 (v tiles; q/k blocks of
            # the next fp8 pair) is independent of this chunk's serial layer
            # chain: queue it and interleave the emission into the layer
            # boundaries to fill PE bubbles
            if fp8_qk0 and c + 1 < n_chunks:
                vTs[c + 1], vem = proj_v_tiles(xts[c + 1], KI, w0v_ap, -1,
                                               name=f"l0c{c + 1}")
                vem = list(vem)
                if c % 2 == 1 and c + 2 < n_chunks:
                    (qks[c + 1], qks[c + 2],
                     qk_em) = proj_qk0_pair(xt8s[c + 1], xt8s[c + 2])
                    # alternate v tiles and qk blocks in the fill queue
                    mixed = []
                    for i in range(max(len(vem), len(qk_em))):
                        if i < len(vem):
                            mixed.append(vem[i])
                        if i < len(qk_em):
                            mixed.append(qk_em[i])
                    fill_q.extend(mixed)
                else:
                    fill_q.extend(vem)
            drain(1)
            for ly in range(2):
                q2, k2, vT2 = proj_layer(x, ly)
                x = attention(q2, k2, vT2, x, fill=lambda: drain(1))
                drain(1)
            q2, k2, vT2 = proj_layer(x, 2)
            attention_last(q2, k2, vT2, x, c, fill=lambda: drain(2))
            drain(len(fill_q))  # rest must land before the next chunk starts
        classifier()

    nc.finalize()
    return nc


def _q8(a):
    return np.clip(np.asarray(a, np.float32), -240.0, 240.0).astype(
        ml_dtypes.float8_e4m3)


def prep_weights(Wq0, bq0, Wk0, bk0, Wv0, bv0, Wq, bq, Wk, bk, Wv, bv,
                 W1, b1, W2, b2, KI=17, use_m=True, fp8_qk0=True):
    """Host-side weight prep shared by all cores."""
    bf = ml_dtypes.bfloat16
    CINP = KI * 128
    w0T = np.zeros((3, CINP, C), np.float32)
    for j, (W_, b_) in enumerate([(Wq0, bq0), (Wk0, bk0), (Wv0, bv0)]):
        w0T[j, :CIN, :] = np.asarray(W_, np.float32).T
        w0T[j, CIN, :] = np.asarray(b_, np.float32)
    w0qk8 = None
    if fp8_qk0:
        # layer-0 q/k weights, x64 and e4m3, padded to 256-row DoubleRow
        # tiles: [2, 128, 2*KI8, C] with (p, ks, c) = 64*W^T[ks*128+p, c]
        KI8 = (KI + 1) // 2
        w8 = np.zeros((2, 2 * KI8 * 128, C), np.float32)
        w8[:, :CINP, :] = w0T[:2] * 64.0
        w0qk8 = _q8(w8.reshape(2, 2 * KI8, 128, C).transpose(0, 2, 1, 3))
    if use_m:
        # lhsT for the fused qk projection: mT[ly] = Wq[ly]^T @ Wk[ly]
        wqT = np.stack([
            (np.asarray(Wq, np.float64)[n].T @ np.asarray(Wk, np.float64)[n])
            .astype(np.float32) for n in range(3)])
    else:
        wqT = np.ascontiguousarray(
            np.transpose(np.asarray(Wq, np.float32), (0, 2, 1)))
    wkT = np.ascontiguousarray(np.transpose(np.asarray(Wk, np.float32), (0, 2, 1)))
    wvT = np.ascontiguousarray(np.transpose(np.asarray(Wv, np.float32), (0, 2, 1)))
    # bqk[p, ct, col]: cols 0..2 = bq layers 1..3, 3..5 = bk layers 1..3
    bqk = np.zeros((128, C // 128, 6), np.float32)
    for ly in range(3):
        bqk[:, :, ly] = np.asarray(bq, np.float32)[ly].reshape(C // 128, 128).T
        bqk[:, :, 3 + ly] = np.asarray(bk, np.float32)[ly].reshape(C // 128, 128).T
    w1T = np.ascontiguousarray(np.asarray(W1, np.float32).T)
    b1h = np.ascontiguousarray(np.asarray(b1, np.float32).reshape(2, 128).T)
    w2T = np.ascontiguousarray(np.asarray(W2, np.float32).T)
    b2h = np.asarray(b2, np.float32).reshape(2, 1)
    KIv = CINP // 128
    CTv = C // 128
    w0T = np.ascontiguousarray(
        w0T.reshape(3, KIv, 128, C).transpose(0, 2, 1, 3))
    wqT = np.ascontiguousarray(
        wqT.reshape(3, CTv, 128, C).transpose(0, 2, 1, 3))
    wkT = np.ascontiguousarray(
        wkT.reshape(3, CTv, 128, C).transpose(0, 2, 1, 3))
    wvT = np.ascontiguousarray(
        wvT.reshape(3, CTv, 128, C).transpose(0, 2, 1, 3))
    w1T = np.ascontiguousarray(w1T.reshape(8, 128, 256).transpose(1, 0, 2))
    out = {
        "w0T": w0T.astype(bf),
        "wvT": wvT.astype(bf),
        "bqk": bqk,
        "bvT": np.asarray(bv, np.float32).astype(bf),
        "w1T": w1T,
        "b1": b1h,
        "w2T": w2T,
        "b2": b2h,
    }
    if use_m:
        out["mT"] = wqT.astype(bf)
    else:
        out["wqT"] = wqT.astype(bf)
        out["wkT"] = wkT.astype(bf)
    if fp8_qk0:
        out["w0qk8"] = w0qk8
    return out


def prep_xT(dl, dr, KI=17, G=8):
    """[n_pairs, L, CIN] left+right -> [n_chunks, 128, KI, G*L] bf16 chunks.

    Sequence order: all left sequences then all right sequences.
    Row CIN is the ones-row that carries layer-0 biases.
    """
    bf = ml_dtypes.bfloat16
    CINP = KI * 128
    n_pairs = dl.shape[0]
    ntok = n_pairs * L
    xT = np.zeros((CINP, 2 * ntok), np.float32)
    xT[:CIN, :ntok] = np.asarray(dl, np.float32).reshape(ntok, CIN).T
    xT[:CIN, ntok:] = np.asarray(dr, np.float32).reshape(ntok, CIN).T
    xT[CIN, :] = 1.0
    T = G * L
    n_chunks = (2 * ntok) // T
    # [CINP, tok] -> [n_chunks, 128, KI, T]
    out = xT.reshape(KI, 128, n_chunks, T).transpose(2, 1, 0, 3)
    return np.ascontiguousarray(out).astype(bf), xT


def prep_xT8(xT, KI=17, G=8):
    """fp8 copy of the token-major input for the DoubleRow q/k path:
    [CINP, tok] -> [n_chunks, 128, 2*KI8, T8] e4m3."""
    KI8 = (KI + 1) // 2
    T = G * L
    T8 = (T + 15) // 16 * 16
    n_chunks = xT.shape[1] // T
    out = np.zeros((2 * KI8 * 128, n_chunks, T8), np.float32)
    out[:xT.shape[0]].reshape(-1, n_chunks, T8)[:, :, :T] = (
        xT.reshape(-1, n_chunks, T))
    out = out.reshape(2 * KI8, 128, n_chunks, T8).transpose(2, 1, 0, 3)
    return _q8(np.ascontiguousarray(out))


def _ensure_ntff_hook():
    """Provide antenv.axon_hooks with a ctypes NTFF profile hook if the
    image's antenv lacks it (bass_utils imports it unguarded when
    trace=True under axon)."""
    try:
        from antenv.axon_hooks import get_axon_ntff_profile_hook  # noqa: F401
        return
    except ImportError:
        pass
    import contextlib
    import ctypes
    import types

    import antenv

    mod = types.ModuleType("antenv.axon_hooks")
    holder = {"hook": None}
    mod.set_axon_ntff_profile_hook = lambda h: holder.update(hook=h)
    mod.get_axon_ntff_profile_hook = lambda: holder["hook"]
    sys.modules["antenv.axon_hooks"] = mod
    antenv.axon_hooks = mod

    so_path = "/opt/axon/libaxon_pjrt.so"
    try:
        lib = ctypes.CDLL(so_path)
    except OSError:
        return
    if not hasattr(lib, "axon_start_nrt_profile"):
        return
    lib.axon_start_nrt_profile.argtypes = [ctypes.POINTER(ctypes.c_int64),
                                           ctypes.c_size_t]
    lib.axon_start_nrt_profile.restype = ctypes.c_int64
    lib.axon_stop_nrt_profile.argtypes = [ctypes.c_char_p]
    lib.axon_stop_nrt_profile.restype = ctypes.c_int64

    @contextlib.contextmanager
    def _hook(output_dir, device_ids):
        import jax

        jax.devices()
        if device_ids:
            ids = (ctypes.c_int64 * len(device_ids))(*device_ids)
            rc = lib.axon_start_nrt_profile(ids, len(device_ids))
        else:
            rc = lib.axon_start_nrt_profile(None, 0)
        if rc != 0:
            raise RuntimeError(f"axon_start_nrt_profile rc={rc}")
        try:
            yield
        finally:
            n = lib.axon_stop_nrt_profile(str(output_dir).encode())
            print(f"ntff profile: {n} file(s) written to {output_dir}",
                  file=sys.stderr)

    holder["hook"] = _hook


_GRAPH_CACHE = {}


def _get_graph(n_seq, G, KI, vbias, use_m, fp8_qk0):
    key = (n_seq, G, KI, vbias, use_m, fp8_qk0)
    if key not in _GRAPH_CACHE:
        _GRAPH_CACHE[key] = build_graph(n_seq=n_seq, G=G, KI=KI, vbias=vbias,
                                        use_m=use_m, fp8_qk0=fp8_qk0)
    return _GRAPH_CACHE[key]


FP8_QK0 = True


def kernel(dataleft, dataright, Wq0, bq0, Wk0, bk0, Wv0, bv0,
           Wq, bq, Wk, bk, Wv, bv, W1, b1, W2, b2):
    import os

    B = dataleft.shape[0]
    per = B // N_CORES
    vbias = bool(np.any(np.asarray(bv)))
    use_m = not (np.any(np.asarray(bq)) or np.any(np.asarray(bk)))
    fp8_qk0 = FP8_QK0
    nc = _get_graph(n_seq=2 * per, G=8, KI=17, vbias=vbias, use_m=use_m,
                    fp8_qk0=fp8_qk0)
    wmap = prep_weights(Wq0, bq0, Wk0, bk0, Wv0, bv0, Wq, bq, Wk, bk, Wv, bv,
                        W1, b1, W2, b2, use_m=use_m, fp8_qk0=fp8_qk0)
    in_maps = []
    for i in range(N_CORES):
        m = dict(wmap)
        m["xT"], xT_full = prep_xT(dataleft[i * per:(i + 1) * per],
                                   dataright[i * per:(i + 1) * per])
        if fp8_qk0:
            m["xT8"] = prep_xT8(xT_full)
        in_maps.append(m)
    trace = bool(int(os.environ.get("KERNEL_TRACE", "0")))
    if trace:
        _ensure_ntff_hook()
    res = run_bass_kernel_spmd(nc, in_maps, core_ids=list(range(N_CORES)),
                               trace=trace)
    if trace and res.exec_time_ns is not None:
        print(f"HW exec time: {res.exec_time_ns} ns")
        kernel.last_exec_time_ns = res.exec_time_ns
        kernel.last_profile = res
    out = np.concatenate([np.ascontiguousarray(r["out"].T) for r in res.results], 0)
    return out.astype(np.float32)

